# revision 23
# baseline (speedup 1.0000x reference)
"""BoundaryLoss kernel for 8 Trainium2 NeuronCores.

loss = sum_c mean_{b,h,w}((|sobel(labels_c)| - |sobel(probs_c)|)^2)
     = sum_sq_err / (B*H*W)

Data-parallel: core k processes batches [2k, 2k+1] x classes 1..4
(8 image pairs of 512x512). Per-core partial sums are combined on host.

On-device pipeline per (pair, row-band):
  - DMA 128-row halo band of labels + probs image (fp32, padded cols).
  - TensorE: gx = Bv @ x[w-1] - Bv @ x[w+1]; gy = Bdf @ (x[w-1] + 2x[w] + x[w+1])
    via 5 float32r band-matrix matmuls per input accumulating in PSUM.
  - ScalarE/VectorE: square PSUM -> fp16, m = gx^2+gy^2, G = sqrt(m+eps),
    e = G_l - G_p, then tensor_tensor_reduce(e*e) -> per-band partial sums.
"""

import sys

import numpy as np

if "/opt/trn_rl_repo" not in sys.path:
    sys.path.insert(0, "/opt/trn_rl_repo")

from contextlib import ExitStack

import concourse.bass as bass
import concourse.mybir as mybir
import concourse.tile as tile

H = W = 512
N_IMG = 8          # image pairs per core
BAND = 126         # output rows per full band
N_BANDS = 4        # full 126-row bands; bottom 8 rows via 2 packed iters
N_ITERS = N_IMG * N_BANDS + 2
PADW = W + 2       # padded columns
SMOOTH = 1e-6
# columns of the 2048-wide PSUM square handled by ScalarE (rest on VectorE)
ACT_SQ_COLS = 1696

F32 = mybir.dt.float32
F32R = mybir.dt.float32r
F16 = mybir.dt.float16


def _band_geom(t):
    """Returns (row0, nrows_loaded, dst_part0, n_valid_out, n_contract)."""
    if t == 0:
        return 0, 127, 1, BAND, 128
    if t < 4:
        r0 = BAND * t
        return r0 - 1, 128, 0, BAND, 128
    # kp=9: row 512 (would be partition 9) is simply dropped from the
    # contraction, which equals the zero-padding contribution.
    return 503, 9, 0, 8, 9


def _stationaries():
    """lhsT weight matrices [p, c]: moving partition p -> out partition c."""
    bv = np.zeros((128, 128), np.float32)   # vertical smooth [1,2,1]
    bdf = np.zeros((128, 128), np.float32)  # vertical diff [1,0,-1]
    for c in range(126):
        bv[c, c] = 1.0
        bv[c + 1, c] = 2.0
        bv[c + 2, c] = 1.0
        bdf[c, c] = 1.0
        bdf[c + 2, c] = -1.0
    # Packed bottom-band versions: 4 images per iteration; image k's rows
    # 503..511 live at input partitions 16k..16k+8 (16k+9 is the zeroed
    # row-512 halo), outputs 504..511 at partitions 8k..8k+7.
    bvm = np.zeros((128, 128), np.float32)
    bdfm = np.zeros((128, 128), np.float32)
    for k in range(4):
        for i in range(8):
            bvm[16 * k + i, 8 * k + i] = 1.0
            bvm[16 * k + i + 1, 8 * k + i] = 2.0
            bvm[16 * k + i + 2, 8 * k + i] = 1.0
            bdfm[16 * k + i, 8 * k + i] = 1.0
            bdfm[16 * k + i + 2, 8 * k + i] = -1.0
    return np.concatenate(
        [bv, -bv, bdf, 2.0 * bdf, bvm, -bvm, bdfm, 2.0 * bdfm], axis=1)


def _split_waits_json(bir: bytes, maxw: int = 1) -> bytes:
    """Walrus in this container rejects instructions with >1 semaphore wait
    ("Too many sync wait commands"). Split extra waits onto NoOp carriers
    inserted just before the instruction on the same engine — semantics are
    identical (same waits, same order, before the instruction executes)."""
    import orjson

    d = orjson.loads(bir)
    ctr = 0
    for fn in d["functions"]:
        for b in fn["blocks"]:
            new = []
            for ins in b["instructions"]:
                si = ins.get("sync_info")
                if si:
                    waits = si.get("on_wait") or []
                    if len(waits) > maxw:
                        keep = waits[-maxw:] if maxw else []
                        for w in waits[: len(waits) - maxw]:
                            ctr += 1
                            new.append({
                                "debug": ins.get("debug", 0),
                                "engine": ins["engine"],
                                "ins": [],
                                "outs": [],
                                "name": f"{ins['name']}-wsplit{ctr}",
                                "opcode": "NoOp",
                                "sync_info": {"on_wait": [w], "on_update": []},
                            })
                        si["on_wait"] = keep
                new.append(ins)
            b["instructions"] = new
    return orjson.dumps(d)


def _patch_serialization(nc):
    fixed = _split_waits_json(nc.to_json_bytes())
    nc.to_json_bytes = lambda: fixed
    return nc


def build_kernel(loop: int = 1):
    nc = bass.Bass()
    labels = nc.dram_tensor("labels", [N_IMG, H, W], F32, kind="ExternalInput")
    probs = nc.dram_tensor("probs", [N_IMG, H, W], F32, kind="ExternalInput")
    consts = nc.dram_tensor("consts", [128, 1024], F32, kind="ExternalInput")
    out = nc.dram_tensor("out", [128, 3], F32, kind="ExternalOutput")

    with ExitStack() as ctx:
        tc = ctx.enter_context(tile.TileContext(nc))
        cpool = ctx.enter_context(tc.tile_pool(name="consts", bufs=1))
        xpool = ctx.enter_context(tc.tile_pool(name="x", bufs=1))
        psum_pool = ctx.enter_context(tc.tile_pool(name="g", bufs=2, space="PSUM"))
        sq_pool = ctx.enter_context(tc.tile_pool(name="sq", bufs=4))
        m_pool = ctx.enter_context(tc.tile_pool(name="m", bufs=4))
        g2_pool = ctx.enter_context(tc.tile_pool(name="G", bufs=4))
        e_pool = ctx.enter_context(tc.tile_pool(name="e", bufs=4))
        esq_pool = ctx.enter_context(tc.tile_pool(name="esq", bufs=4))
        acc_pool = ctx.enter_context(tc.tile_pool(name="acc", bufs=1))

        wmat = cpool.tile([128, 1024], F32, tag="wmat")
        nc.sync.dma_start(out=wmat[:, :], in_=consts[:, :])
        wr = cpool.tile([128, 1024], F32R, tag="wr")
        nc.vector.tensor_copy(wr[:, :], wmat[:, :])
        (BV, BVN, BDF, BDF2, BVM, BVNM, BDFM, BDF2M) = (
            wr[:, 128 * i:128 * i + 128] for i in range(8))

        acc_a = acc_pool.tile([128, N_ITERS], F32, tag="acc_a")
        acc_b = acc_pool.tile([128, N_ITERS], F32, tag="acc_b")
        acc_c = acc_pool.tile([128, N_ITERS], F32, tag="acc_c")
        nc.vector.memset(acc_a[:, :], 0.0)
        nc.vector.memset(acc_b[:, :], 0.0)
        nc.vector.memset(acc_c[:, :], 0.0)
        out_s = acc_pool.tile([128, 3], F32, tag="out_s")

        # 8 persistent x tiles; band t always lands on tiles {2t, 2t+1}.
        # Pad regions are zeroed once and never overwritten (the DMAs fill
        # the interior only).
        xt = [xpool.tile([128, PADW], F32, name=f"x{j}", tag=f"x{j}")
              for j in range(8)]
        xrt = [xpool.tile([128, PADW], F32R, name=f"xr{j}", tag=f"xr{j}")
               for j in range(8)]
        for j in range(8):
            nc.vector.memset(xt[j][:, 0:1], 0.0)
            nc.vector.memset(xt[j][:, PADW - 1:PADW], 0.0)
        for j in (0, 1):
            nc.vector.memset(xt[j][0:1, :], 0.0)   # top band: row -1
        # 4 tiles for the packed bottom-band iterations (2 per input side).
        xm = [xpool.tile([128, PADW], F32, name=f"xm{j}", tag=f"xm{j}")
              for j in range(4)]
        xmr = [xpool.tile([128, PADW], F32R, name=f"xmr{j}", tag=f"xmr{j}")
               for j in range(4)]
        for j in range(4):
            nc.vector.memset(xm[j][0:64, :], 0.0)

        loop_ctx = tc.For_i(0, loop, 1) if loop > 1 else None
        if loop_ctx is not None:
            loop_ctx.__enter__()

        def emit_mms(g, xlr, xpr, stat, pv, kp):
            sv, svn, sdf, sdf2 = stat
            for j, x in enumerate((xlr, xpr)):
                cx, cy = 1024 * j, 1024 * j + 512
                nc.tensor.matmul(g[0:pv, cx:cx + 512], sv[0:kp, 0:pv],
                                 x[0:kp, 0:W], start=True, stop=False)
                nc.tensor.matmul(g[0:pv, cx:cx + 512], svn[0:kp, 0:pv],
                                 x[0:kp, 2:2 + W], start=False, stop=True)
                nc.tensor.matmul(g[0:pv, cy:cy + 512], sdf[0:kp, 0:pv],
                                 x[0:kp, 0:W], start=True, stop=False)
                nc.tensor.matmul(g[0:pv, cy:cy + 512], sdf[0:kp, 0:pv],
                                 x[0:kp, 2:2 + W], start=False, stop=False)
                nc.tensor.matmul(g[0:pv, cy:cy + 512], sdf2[0:kp, 0:pv],
                                 x[0:kp, 1:1 + W], start=False, stop=True)

        it = 0
        for phase in range(N_IMG + 2):
            if phase < N_IMG:
                img = phase
                bands = range(N_BANDS)
            else:
                bands = (-1,)
            for t in bands:
                if t >= 0:
                    r0, nrows, p0, pv, kp = _band_geom(t)
                    xl, xp_ = xt[2 * t], xt[2 * t + 1]
                    xlr, xpr = xrt[2 * t], xrt[2 * t + 1]
                    nc.sync.dma_start(
                        out=xl[p0:p0 + nrows, 1:1 + W],
                        in_=labels[img, r0:r0 + nrows, :])
                    nc.sync.dma_start(
                        out=xp_[p0:p0 + nrows, 1:1 + W],
                        in_=probs[img, r0:r0 + nrows, :])
                    nc.vector.tensor_copy(xlr[0:p0 + nrows, :],
                                          xl[0:p0 + nrows, :])
                    nc.vector.tensor_copy(xpr[0:p0 + nrows, :],
                                          xp_[0:p0 + nrows, :])
                    stat, pv, kp = (BV, BVN, BDF, BDF2), BAND, 128
                else:
                    # Packed bottom bands: rows 503..511 of 4 images.
                    q = phase - N_IMG
                    xl, xp_ = xm[2 * q], xm[2 * q + 1]
                    xlr, xpr = xmr[2 * q], xmr[2 * q + 1]
                    for k in range(4):
                        img_k = 4 * q + k
                        nc.sync.dma_start(
                            out=xl[16 * k:16 * k + 9, 1:1 + W],
                            in_=labels[img_k, 503:512, :])
                        nc.sync.dma_start(
                            out=xp_[16 * k:16 * k + 9, 1:1 + W],
                            in_=probs[img_k, 503:512, :])
                    nc.vector.tensor_copy(xlr[0:64, :], xl[0:64, :])
                    nc.vector.tensor_copy(xpr[0:64, :], xp_[0:64, :])
                    stat, pv, kp = (BVM, BVNM, BDFM, BDF2M), 32, 58

                # PSUM layout: [gx_l | gy_l | gx_p | gy_p], 512 f32 each.
                g = psum_pool.tile([128, 2048], F32)
                emit_mms(g, xlr, xpr, stat, pv, kp)

                # Squares of all four gradients, PSUM -> SBUF fp16. DVE
                # cannot read two PSUM operands in one op, so its share goes
                # through an fp16 copy. Sum(gx^2+gy^2) over both inputs is
                # captured for free by the accum_out of the ACT square and
                # the DVE TTR square. (SMOOTH inside the sqrt contributes
                # ~1e-7 relative to the loss and is dropped.)
                sq = sq_pool.tile([128, 2048], F16)
                nc.scalar.activation(sq[0:pv, 0:ACT_SQ_COLS],
                                     g[0:pv, 0:ACT_SQ_COLS],
                                     mybir.ActivationFunctionType.Square,
                                     accum_out=acc_a[0:pv, it:it + 1])
                dc = 2048 - ACT_SQ_COLS
                c16 = e_pool.tile([128, dc], F16)
                nc.vector.tensor_copy(c16[0:pv, :], g[0:pv, ACT_SQ_COLS:2048])
                nc.vector.scalar_tensor_tensor(
                    out=sq[0:pv, ACT_SQ_COLS:2048], in0=c16[0:pv, :],
                    scalar=1.0, in1=c16[0:pv, :],
                    op0=mybir.AluOpType.mult, op1=mybir.AluOpType.mult,
                    accum_out=acc_c[0:pv, it:it + 1])

                # m = gx^2 + gy^2 for both inputs: [m_l | m_p]
                m = m_pool.tile([128, 1024], F16)
                sqv = sq.rearrange("p (a b c) -> p a b c", a=2, b=2, c=512)
                mv = m.rearrange("p (a c) -> p a c", a=2, c=512)
                nc.vector.tensor_add(mv[0:pv, :, :], sqv[0:pv, :, 0, :],
                                     sqv[0:pv, :, 1, :])

                # (G_l - G_p)^2 = m_l + m_p - 2*sqrt(m_l * m_p)
                qp = g2_pool.tile([128, 512], F16)
                nc.vector.tensor_mul(qp[0:pv, :], m[0:pv, 0:512], m[0:pv, 512:1024])
                s = esq_pool.tile([128, 512], F16)
                nc.scalar.activation(s[0:pv, :], qp[0:pv, :],
                                     mybir.ActivationFunctionType.Sqrt,
                                     accum_out=acc_b[0:pv, it:it + 1])
                it += 1

        if loop_ctx is not None:
            loop_ctx.__exit__(None, None, None)
        nc.vector.tensor_reduce(out_s[:, 0:1], acc_a[:, :],
                                axis=mybir.AxisListType.X, op=mybir.AluOpType.add)
        nc.vector.tensor_reduce(out_s[:, 1:2], acc_b[:, :],
                                axis=mybir.AxisListType.X, op=mybir.AluOpType.add)
        nc.vector.tensor_reduce(out_s[:, 2:3], acc_c[:, :],
                                axis=mybir.AxisListType.X, op=mybir.AluOpType.add)
        nc.sync.dma_start(out=out[:, :], in_=out_s[:, :])
    return _patch_serialization(nc)


_NC = None


def kernel(probs, labels):
    global _NC
    from concourse.bass_utils import run_bass_kernel_spmd

    if _NC is None:
        _NC = build_kernel()

    p = np.ascontiguousarray(np.asarray(probs)[:, 1:5].astype(np.float32, copy=False))
    l = np.ascontiguousarray(np.asarray(labels)[:, 1:5].astype(np.float32, copy=False))
    wmat = _stationaries()

    in_maps = []
    for k in range(8):
        in_maps.append({
            "probs": np.ascontiguousarray(p[2 * k:2 * k + 2].reshape(N_IMG, H, W)),
            "labels": np.ascontiguousarray(l[2 * k:2 * k + 2].reshape(N_IMG, H, W)),
            "consts": wmat,
        })
    res = run_bass_kernel_spmd(_NC, in_maps, list(range(8)))
    total = 0.0
    for r in res.results:
        o = r["out"].astype(np.float64)
        total += o[:, 0].sum() + o[:, 2].sum() - 2.0 * o[:, 1].sum()
    return np.float32(total / (16 * H * W))


# revision 25
# speedup vs baseline: 1.5460x; 1.5460x over previous
"""BoundaryLoss kernel for 8 Trainium2 NeuronCores.

loss = sum_c mean_{b,h,w}((|sobel(labels_c)| - |sobel(probs_c)|)^2)
     = sum_sq_err / (B*H*W)

Data-parallel: core k processes batches [2k, 2k+1] x classes 1..4
(8 image pairs of 512x512). Per-core partial sums are combined on host.

On-device pipeline per (pair, row-band):
  - DMA 128-row halo band of labels + probs image (fp32, padded cols).
  - TensorE: gx = Bv @ x[w-1] - Bv @ x[w+1]; gy = Bdf @ (x[w-1] + 2x[w] + x[w+1])
    via 5 float32r band-matrix matmuls per input accumulating in PSUM.
  - ScalarE/VectorE: square PSUM -> fp16, m = gx^2+gy^2, G = sqrt(m+eps),
    e = G_l - G_p, then tensor_tensor_reduce(e*e) -> per-band partial sums.
"""

import sys

import numpy as np

if "/opt/trn_rl_repo" not in sys.path:
    sys.path.insert(0, "/opt/trn_rl_repo")

from contextlib import ExitStack

import concourse.bass as bass
import concourse.mybir as mybir
import concourse.tile as tile

H = W = 512
N_IMG = 8          # image pairs per core
BAND = 126         # output rows per full band
N_BANDS = 4        # full 126-row bands; bottom 8 rows via 2 packed iters
N_ITERS = N_IMG * N_BANDS + 2
PADW = W + 2       # padded columns
SMOOTH = 1e-6
# columns of the 2048-wide PSUM square handled by ScalarE (rest on VectorE)
ACT_SQ_COLS = 1696

F32 = mybir.dt.float32
F32R = mybir.dt.float32r
F16 = mybir.dt.float16


def _band_geom(t):
    """Returns (row0, nrows_loaded, dst_part0, n_valid_out, n_contract)."""
    if t == 0:
        return 0, 127, 1, BAND, 128
    if t < 4:
        r0 = BAND * t
        return r0 - 1, 128, 0, BAND, 128
    # kp=9: row 512 (would be partition 9) is simply dropped from the
    # contraction, which equals the zero-padding contribution.
    return 503, 9, 0, 8, 9


def _stationaries():
    """lhsT weight matrices [p, c]: moving partition p -> out partition c."""
    bv = np.zeros((128, 128), np.float32)   # vertical smooth [1,2,1]
    bdf = np.zeros((128, 128), np.float32)  # vertical diff [1,0,-1]
    for c in range(126):
        bv[c, c] = 1.0
        bv[c + 1, c] = 2.0
        bv[c + 2, c] = 1.0
        bdf[c, c] = 1.0
        bdf[c + 2, c] = -1.0
    # Packed bottom-band versions: 4 images per iteration; image k's rows
    # 503..511 live at input partitions 16k..16k+8 (16k+9 is the zeroed
    # row-512 halo), outputs 504..511 at partitions 8k..8k+7.
    bvm = np.zeros((128, 128), np.float32)
    bdfm = np.zeros((128, 128), np.float32)
    for k in range(4):
        for i in range(8):
            bvm[16 * k + i, 8 * k + i] = 1.0
            bvm[16 * k + i + 1, 8 * k + i] = 2.0
            bvm[16 * k + i + 2, 8 * k + i] = 1.0
            bdfm[16 * k + i, 8 * k + i] = 1.0
            bdfm[16 * k + i + 2, 8 * k + i] = -1.0
    return np.concatenate(
        [bv, -bv, bdf, 2.0 * bdf, bvm, -bvm, bdfm, 2.0 * bdfm],
        axis=1).astype(np.float16)


def _split_waits_json(bir: bytes, maxw: int = 1) -> bytes:
    """Walrus in this container rejects instructions with >1 semaphore wait
    ("Too many sync wait commands"). Split extra waits onto NoOp carriers
    inserted just before the instruction on the same engine — semantics are
    identical (same waits, same order, before the instruction executes)."""
    import orjson

    d = orjson.loads(bir)
    ctr = 0
    for fn in d["functions"]:
        for b in fn["blocks"]:
            new = []
            for ins in b["instructions"]:
                si = ins.get("sync_info")
                if si:
                    waits = si.get("on_wait") or []
                    if len(waits) > maxw:
                        keep = waits[-maxw:] if maxw else []
                        for w in waits[: len(waits) - maxw]:
                            ctr += 1
                            new.append({
                                "debug": ins.get("debug", 0),
                                "engine": ins["engine"],
                                "ins": [],
                                "outs": [],
                                "name": f"{ins['name']}-wsplit{ctr}",
                                "opcode": "NoOp",
                                "sync_info": {"on_wait": [w], "on_update": []},
                            })
                        si["on_wait"] = keep
                new.append(ins)
            b["instructions"] = new
    return orjson.dumps(d)


def _patch_serialization(nc):
    fixed = _split_waits_json(nc.to_json_bytes())
    nc.to_json_bytes = lambda: fixed
    return nc


def build_kernel(loop: int = 1):
    nc = bass.Bass()
    labels = nc.dram_tensor("labels", [N_IMG, H, W], F16, kind="ExternalInput")
    probs = nc.dram_tensor("probs", [N_IMG, H, W], F16, kind="ExternalInput")
    consts = nc.dram_tensor("consts", [128, 1024], F16, kind="ExternalInput")
    out = nc.dram_tensor("out", [128, 3], F32, kind="ExternalOutput")

    with ExitStack() as ctx:
        tc = ctx.enter_context(tile.TileContext(nc))
        cpool = ctx.enter_context(tc.tile_pool(name="consts", bufs=1))
        xpool = ctx.enter_context(tc.tile_pool(name="x", bufs=1))
        psum_pool = ctx.enter_context(tc.tile_pool(name="g", bufs=2, space="PSUM"))
        sq_pool = ctx.enter_context(tc.tile_pool(name="sq", bufs=4))
        m_pool = ctx.enter_context(tc.tile_pool(name="m", bufs=4))
        g2_pool = ctx.enter_context(tc.tile_pool(name="G", bufs=4))
        e_pool = ctx.enter_context(tc.tile_pool(name="e", bufs=4))
        esq_pool = ctx.enter_context(tc.tile_pool(name="esq", bufs=4))
        acc_pool = ctx.enter_context(tc.tile_pool(name="acc", bufs=1))

        wmat = cpool.tile([128, 1024], F16, tag="wmat")
        nc.sync.dma_start(out=wmat[:, :], in_=consts[:, :])
        (BV, BVN, BDF, BDF2, BVM, BVNM, BDFM, BDF2M) = (
            wmat[:, 128 * i:128 * i + 128] for i in range(8))

        acc_a = acc_pool.tile([128, N_ITERS], F32, tag="acc_a")
        acc_b = acc_pool.tile([128, N_ITERS], F32, tag="acc_b")
        acc_c = acc_pool.tile([128, N_ITERS], F32, tag="acc_c")
        nc.vector.memset(acc_a[:, :], 0.0)
        nc.vector.memset(acc_b[:, :], 0.0)
        nc.vector.memset(acc_c[:, :], 0.0)
        out_s = acc_pool.tile([128, 3], F32, tag="out_s")

        # 8 persistent x tiles; band t always lands on tiles {2t, 2t+1}.
        # Pad regions are zeroed once and never overwritten (the DMAs fill
        # the interior only).
        xt = [xpool.tile([128, PADW], F16, name=f"x{j}", tag=f"x{j}")
              for j in range(8)]
        for j in range(8):
            nc.vector.memset(xt[j][:, 0:1], 0.0)
            nc.vector.memset(xt[j][:, PADW - 1:PADW], 0.0)
        for j in (0, 1):
            nc.vector.memset(xt[j][0:1, :], 0.0)   # top band: row -1
        # 4 tiles for the packed bottom-band iterations (2 per input side).
        xm = [xpool.tile([128, PADW], F16, name=f"xm{j}", tag=f"xm{j}")
              for j in range(4)]
        for j in range(4):
            nc.vector.memset(xm[j][0:64, :], 0.0)

        loop_ctx = tc.For_i(0, loop, 1) if loop > 1 else None
        if loop_ctx is not None:
            loop_ctx.__enter__()

        def emit_mms(g, xlr, xpr, stat, pv, kp):
            # Stationary-major order: 4 weight loads per iteration, not 10.
            sv, svn, sdf, sdf2 = stat
            xs = ((xlr, 0), (xpr, 1024))
            for x, c in xs:
                nc.tensor.matmul(g[0:pv, c:c + 512], sv[0:kp, 0:pv],
                                 x[0:kp, 0:W], start=True, stop=False)
            for x, c in xs:
                nc.tensor.matmul(g[0:pv, c:c + 512], svn[0:kp, 0:pv],
                                 x[0:kp, 2:2 + W], start=False, stop=True)
            for x, c in xs:
                nc.tensor.matmul(g[0:pv, c + 512:c + 1024], sdf[0:kp, 0:pv],
                                 x[0:kp, 0:W], start=True, stop=False)
                nc.tensor.matmul(g[0:pv, c + 512:c + 1024], sdf[0:kp, 0:pv],
                                 x[0:kp, 2:2 + W], start=False, stop=False)
            for x, c in xs:
                nc.tensor.matmul(g[0:pv, c + 512:c + 1024], sdf2[0:kp, 0:pv],
                                 x[0:kp, 1:1 + W], start=False, stop=True)

        it = 0
        for phase in range(N_IMG + 2):
            if phase < N_IMG:
                img = phase
                bands = range(N_BANDS)
            else:
                bands = (-1,)
            for t in bands:
                if t >= 0:
                    r0, nrows, p0, pv, kp = _band_geom(t)
                    xlr, xpr = xt[2 * t], xt[2 * t + 1]
                    nc.sync.dma_start(
                        out=xlr[p0:p0 + nrows, 1:1 + W],
                        in_=labels[img, r0:r0 + nrows, :])
                    nc.sync.dma_start(
                        out=xpr[p0:p0 + nrows, 1:1 + W],
                        in_=probs[img, r0:r0 + nrows, :])
                    stat, pv, kp = (BV, BVN, BDF, BDF2), BAND, 128
                else:
                    # Packed bottom bands: rows 503..511 of 4 images.
                    q = phase - N_IMG
                    xlr, xpr = xm[2 * q], xm[2 * q + 1]
                    for k in range(4):
                        img_k = 4 * q + k
                        nc.sync.dma_start(
                            out=xlr[16 * k:16 * k + 9, 1:1 + W],
                            in_=labels[img_k, 503:512, :])
                        nc.sync.dma_start(
                            out=xpr[16 * k:16 * k + 9, 1:1 + W],
                            in_=probs[img_k, 503:512, :])
                    stat, pv, kp = (BVM, BVNM, BDFM, BDF2M), 32, 58

                # PSUM layout: [gx_l | gy_l | gx_p | gy_p], 512 f32 each.
                g = psum_pool.tile([128, 2048], F32)
                emit_mms(g, xlr, xpr, stat, pv, kp)

                # Squares of all four gradients, PSUM -> SBUF fp16. DVE
                # cannot read two PSUM operands in one op, so its share goes
                # through an fp16 copy. Sum(gx^2+gy^2) over both inputs is
                # captured for free by the accum_out of the ACT square and
                # the DVE TTR square. (SMOOTH inside the sqrt contributes
                # ~1e-7 relative to the loss and is dropped.)
                sq = sq_pool.tile([128, 2048], F16)
                nc.scalar.activation(sq[0:pv, 0:ACT_SQ_COLS],
                                     g[0:pv, 0:ACT_SQ_COLS],
                                     mybir.ActivationFunctionType.Square,
                                     accum_out=acc_a[0:pv, it:it + 1])
                dc = 2048 - ACT_SQ_COLS
                c16 = e_pool.tile([128, dc], F16)
                nc.vector.tensor_copy(c16[0:pv, :], g[0:pv, ACT_SQ_COLS:2048])
                nc.vector.scalar_tensor_tensor(
                    out=sq[0:pv, ACT_SQ_COLS:2048], in0=c16[0:pv, :],
                    scalar=1.0, in1=c16[0:pv, :],
                    op0=mybir.AluOpType.mult, op1=mybir.AluOpType.mult,
                    accum_out=acc_c[0:pv, it:it + 1])

                # m = gx^2 + gy^2 for both inputs: [m_l | m_p]
                m = m_pool.tile([128, 1024], F16)
                sqv = sq.rearrange("p (a b c) -> p a b c", a=2, b=2, c=512)
                mv = m.rearrange("p (a c) -> p a c", a=2, c=512)
                nc.vector.tensor_add(mv[0:pv, :, :], sqv[0:pv, :, 0, :],
                                     sqv[0:pv, :, 1, :])

                # (G_l - G_p)^2 = m_l + m_p - 2*sqrt(m_l * m_p)
                qp = g2_pool.tile([128, 512], F16)
                nc.vector.tensor_mul(qp[0:pv, :], m[0:pv, 0:512], m[0:pv, 512:1024])
                s = esq_pool.tile([128, 512], F16)
                nc.scalar.activation(s[0:pv, :], qp[0:pv, :],
                                     mybir.ActivationFunctionType.Sqrt,
                                     accum_out=acc_b[0:pv, it:it + 1])
                it += 1

        if loop_ctx is not None:
            loop_ctx.__exit__(None, None, None)
        nc.vector.tensor_reduce(out_s[:, 0:1], acc_a[:, :],
                                axis=mybir.AxisListType.X, op=mybir.AluOpType.add)
        nc.vector.tensor_reduce(out_s[:, 1:2], acc_b[:, :],
                                axis=mybir.AxisListType.X, op=mybir.AluOpType.add)
        nc.vector.tensor_reduce(out_s[:, 2:3], acc_c[:, :],
                                axis=mybir.AxisListType.X, op=mybir.AluOpType.add)
        nc.sync.dma_start(out=out[:, :], in_=out_s[:, :])
    return _patch_serialization(nc)


_NC = None


def kernel(probs, labels):
    global _NC
    from concourse.bass_utils import run_bass_kernel_spmd

    if _NC is None:
        _NC = build_kernel()

    p = np.ascontiguousarray(np.asarray(probs)[:, 1:5]).astype(np.float16)
    l = np.ascontiguousarray(np.asarray(labels)[:, 1:5]).astype(np.float16)
    wmat = _stationaries()

    in_maps = []
    for k in range(8):
        in_maps.append({
            "probs": np.ascontiguousarray(p[2 * k:2 * k + 2].reshape(N_IMG, H, W)),
            "labels": np.ascontiguousarray(l[2 * k:2 * k + 2].reshape(N_IMG, H, W)),
            "consts": wmat,
        })
    res = run_bass_kernel_spmd(_NC, in_maps, list(range(8)))
    total = 0.0
    for r in res.results:
        o = r["out"].astype(np.float64)
        total += o[:, 0].sum() + o[:, 2].sum() - 2.0 * o[:, 1].sum()
    return np.float32(total / (16 * H * W))
